# revision 12
# baseline (speedup 1.0000x reference)
"""Trainium2 Bass kernel for a 2-layer leaky-integrate-and-fire SNN.

Model (per timestep t, snnTorch Leaky with reset-by-subtraction):
    cur1 = x_t @ w1.T + b1
    mem1 = beta*mem1_prev + cur1 - (mem1_prev > 1)          # threshold 1.0
    spk1 = (mem1 > 1)
    cur2 = spk1 @ w2.T + b2
    mem2 = beta*mem2_prev + cur2 - (mem2_prev > 1)
    spk2 = (mem2 > 1)
Outputs: spk2 (B,T,O) and mem2 (B,T,O).

Strategy (data-parallel over batch, 16 rows per core):
  * cur1 for ALL timesteps is a feed-forward GEMM (the recurrence is only
    elementwise).  It runs in fp16 with an error-compensated 3-term split
    x@w = xh@wh + xh@wl + xl@wh, xh = fp16(x), xl = fp16(x - xh): fp16 has
    an 11-bit significand (same as tf32) at 1 col/cycle, the PE keeps fp16
    subnormals (HW-verified), and fp16 allows N=1024 moving operands plus
    fast weight load.  CPU-simulated: 0 spike flips vs the f32 reference.
  * GEMM1 runs in 512-column blocks (N=512 is the ISA cap on matmul
    moving-operand elements; N=1024 fails the s3d3_mm_num_elements check).
  * The scan runs on the Vector engine with a scaled state M = beta*mem:
        A:  M_t = (V_{t-1} * -beta) + beta*cur_t   (stt, in-place into c1)
        B:  V_t = (M_t > beta) - M_t               (stt)
    scan1(s) and scan2(s-1) are interleaved op-by-op in the DVE queue so
    each chain's completion-semaphore wait hides behind the other chain.
  * Spikes (0/1, exact in fp16) are extracted blockwise on the DVE, one
    tensor_scalar per h-chunk per sub-block, into fp16 tiles.  (GpSimd is
    useless here: its strided tensor_scalar ran at ~9 G elem/s and starved
    concurrent DVE ops 35x.)
  * Layer-2: per h-chunk the fp16 stationary packs w2h at cols 0..9 and
    w2l at cols 32..41 (PSUM reads need 32-aligned starts) -> 8 matmuls
    per sub-block; strips PSUM[0:10] + PSUM[32:42] combine at eviction.
    w2l is subnormal in fp16; the PE keeps subnormals (HW-verified).
  * mem2 comes back as beta*mem2; the 1/beta un-scale happens on host.
  * Output DMAs are emitted two sub-blocks late so they never block input
    x DMA issue in the in-order Sync queue.

Sub-block layout: C1 tile (128, 32, 8, 16): partition p, local time t,
h-chunk c (h = c*128 + p), batch b.  Scan slices C1[:, t] are contiguous
(128, 128); GEMM1 evictions write strided; spikes go to (128, 8, 32, 16).
"""

import numpy as np

BETA = 0.95
B, T, I, H, O = 128, 200, 784, 1024, 10
NCORES = 8
BL = B // NCORES          # 16 batch rows per core
TB = T * BL               # 3200 (t-major, b-minor columns)
HC = H // 128             # 8 h-chunks
TBLK = 32                 # timesteps per sub-block
CHUNK = TBLK * BL         # 512 columns per sub-block
PBLOCKS = (512, 512, 512, 512, 512, 512, 128)   # physical gemm1 blocks
W2W = 32 + O              # w2h at cols 0..9, w2l at cols 32..41

_nc_cache = None


def _build():
    import concourse.bacc as bacc
    import concourse.mybir as mybir
    from concourse.masks import make_identity as _make_identity
    from concourse.tile import TileContext

    Alu = mybir.AluOpType
    Act = mybir.ActivationFunctionType
    f32 = mybir.dt.float32
    f16 = mybir.dt.float16

    nc = bacc.Bacc("TRN2", target_bir_lowering=False, debug=False)

    KF = 6                # full 128-row contraction chunks (rows 0..767)
    KT = 48               # packed tail: [xh_t; xh_t; xl_t] x [w1h_t; w1l_t; w1h_t]
    xh_d = nc.dram_tensor("xh", (KF * 128, TB), f16, kind="ExternalInput")
    xl_d = nc.dram_tensor("xl", (KF * 128, TB), f16, kind="ExternalInput")
    xt_d = nc.dram_tensor("xt", (KT, TB), f16, kind="ExternalInput")
    w1h_d = nc.dram_tensor("w1h", (KF * 128, H), f16, kind="ExternalInput")
    w1l_d = nc.dram_tensor("w1l", (KF * 128, H), f16, kind="ExternalInput")
    w1t_d = nc.dram_tensor("w1t", (KT, H), f16, kind="ExternalInput")
    b1c = nc.dram_tensor("b1c", (128, HC), f32, kind="ExternalInput")
    w2p_d = nc.dram_tensor("w2p", (128, HC * W2W), f16, kind="ExternalInput")
    b2c = nc.dram_tensor("b2c", (O, 1), f32, kind="ExternalInput")
    S2 = nc.dram_tensor("S2", (O, TB), f32, kind="ExternalOutput")
    M2 = nc.dram_tensor("M2", (O, TB), f32, kind="ExternalOutput")

    # physical blocks (gemm1 granularity)
    pblocks = []
    c0 = 0
    for n in PBLOCKS:
        pblocks.append((c0, n))
        c0 += n
    assert c0 == TB
    # sub-blocks (scan / gemm2 granularity), and phys -> subs map
    blocks = []
    p2s = []
    for (c0, n) in pblocks:
        subs = []
        off = 0
        while off < n:
            m = min(CHUNK, n - off)
            subs.append(len(blocks))
            blocks.append((c0 + off, m))
            off += m
        p2s.append(subs)
    NSUB = len(blocks)

    with TileContext(nc) as tc:
        with (
            tc.tile_pool(name="const", bufs=1) as cpool,
            tc.tile_pool(name="l2", bufs=1) as l2pool,
            tc.tile_pool(name="c1b", bufs=3) as c1pool,
            tc.tile_pool(name="xt", bufs=2) as xpool,
            tc.tile_pool(name="mv", bufs=2) as mvpool,
            tc.tile_pool(name="ps1", bufs=4, space="PSUM") as ps1,
            tc.tile_pool(name="ps2", bufs=2, space="PSUM") as ps2,
        ):
            # Weight DMAs are split per k-chunk: the first 256 w-cols of
            # each chunk ship first (the launch transient runs m=0,1
            # k-outer), remainders follow with a one-chunk lag so the
            # m>=2 groups never wait on them.
            w1h_sb = cpool.tile([128, KF, H], f16)
            w1l_sb = cpool.tile([128, KF, H], f16)
            NP0 = PBLOCKS[0]
            xh0 = xpool.tile([128, KF, CHUNK], f16, tag="xh", name="xh0")
            xl0 = xpool.tile([128, KF, CHUNK], f16, tag="xl", name="xl0")
            xt0 = xpool.tile([KT, CHUNK], f16, tag="xt", name="xt0")
            for k in range(KF):
                nc.sync.dma_start(
                    out=w1h_sb[:, k, 0:256],
                    in_=w1h_d[k * 128:(k + 1) * 128, 0:256],
                )
                nc.sync.dma_start(
                    out=xh0[:, k, :NP0], in_=xh_d[k * 128:(k + 1) * 128, 0:NP0]
                )
                nc.sync.dma_start(
                    out=w1l_sb[:, k, 0:256],
                    in_=w1l_d[k * 128:(k + 1) * 128, 0:256],
                )
                nc.sync.dma_start(
                    out=xl0[:, k, :NP0], in_=xl_d[k * 128:(k + 1) * 128, 0:NP0]
                )
                if k >= 1:
                    kk = k - 1
                    nc.sync.dma_start(
                        out=w1h_sb[:, kk, 256:H],
                        in_=w1h_d[kk * 128:(kk + 1) * 128, 256:H],
                    )
                    nc.sync.dma_start(
                        out=w1l_sb[:, kk, 256:H],
                        in_=w1l_d[kk * 128:(kk + 1) * 128, 256:H],
                    )
            nc.sync.dma_start(out=xt0[:, :NP0], in_=xt_d[:, 0:NP0])
            w1t_sb = cpool.tile([KT, H], f16)
            nc.sync.dma_start(out=w1t_sb[:], in_=w1t_d[:])
            kk = KF - 1
            nc.sync.dma_start(
                out=w1h_sb[:, kk, 256:H],
                in_=w1h_d[kk * 128:(kk + 1) * 128, 256:H],
            )
            nc.sync.dma_start(
                out=w1l_sb[:, kk, 256:H],
                in_=w1l_d[kk * 128:(kk + 1) * 128, 256:H],
            )
            b1_sb = cpool.tile([128, HC], f32)
            nc.sync.dma_start(out=b1_sb[:], in_=b1c[:])
            w2p_sb = cpool.tile([128, HC, W2W], f16)
            b2_sb = cpool.tile([O, 1], f32)

            c2 = l2pool.tile([O, TB], f32)            # beta*cur2 -> beta*mem2 (in place)

            ident = cpool.tile([128, 128], f32)
            _make_identity(nc, ident[:])
            ttmp = cpool.tile([128, H], f32)

            v1 = mvpool.tile([128, HC * BL], f32, tag="v1")
            nc.vector.memset(v1[:], 0.0)
            v2 = mvpool.tile([O, BL], f32, tag="v2")
            nc.vector.memset(v2[:], 0.0)

            c1_tiles = {}
            spk_tiles = {}
            pending_out = {}
            extract_ranges = {}

            def gemm1(p):
                # one physical block: N = 512, 1024, or 128 (flip mode)
                c0, n = pblocks[p]
                if p == 0:
                    xh, xl, xt = xh0, xl0, xt0
                else:
                    xh = xpool.tile([128, KF, CHUNK], f16, tag="xh")
                    xl = xpool.tile([128, KF, CHUNK], f16, tag="xl")
                    xt = xpool.tile([KT, CHUNK], f16, tag="xt")
                    for k in range(KF):
                        nc.sync.dma_start(
                            out=xh[:, k, :n],
                            in_=xh_d[k * 128:(k + 1) * 128, c0:c0 + n],
                        )
                        nc.sync.dma_start(
                            out=xl[:, k, :n],
                            in_=xl_d[k * 128:(k + 1) * 128, c0:c0 + n],
                        )
                    nc.sync.dma_start(out=xt[:, :n], in_=xt_d[:, c0:c0 + n])
                # c1/spk tiles for each sub-block of this physical block
                subs = p2s[p]
                for s in subs:
                    c1_tiles[s] = c1pool.tile(
                        [128, TBLK, HC, BL], f32, tag="c1", name="c1"
                    )
                    spk_tiles[s] = c1pool.tile(
                        [128, HC, TBLK, BL], f16, tag="spk", name="spk"
                    )

                def evict1(p1, m):
                    # one ACT eviction per sub-block covered by this psum
                    for si, s in enumerate(subs):
                        sc0, sn = blocks[s]
                        nt = sn // BL
                        p1v = p1[:, si * CHUNK:si * CHUNK + sn].rearrange(
                            "p (t b) -> p t b", b=BL
                        )
                        nc.scalar.activation(
                            out=c1_tiles[s][:, :nt, m, :],
                            in_=p1v[:, :nt, :],
                            func=Act.Identity,
                            bias=b1_sb[:, m:m + 1],
                            scale=1.0,
                        )

                def m_block(m):
                    p1 = ps1.tile([128, CHUNK], f32, tag="p1")
                    i = 0
                    for k in range(KF):
                        for (wt, xs_) in (
                            (w1h_sb, xh), (w1l_sb, xh), (w1h_sb, xl),
                        ):
                            nc.tensor.matmul(
                                p1[:, :n],
                                lhsT=wt[:, k, m * 128:(m + 1) * 128],
                                rhs=xs_[:, k, :n],
                                start=(i == 0),
                                stop=False,
                            )
                            i += 1
                    nc.tensor.matmul(
                        p1[:, :n],
                        lhsT=w1t_sb[:, m * 128:(m + 1) * 128],
                        rhs=xt[:, :n],
                        start=False,
                        stop=True,
                    )
                    evict1(p1, m)

                if n > 128:
                    if p == 0:
                        # Launch transient: run m=0,1 together k-outer (2x
                        # work per arriving x chunk) so the PE stays busy
                        # from chunk 0 and the HAM clock ramps early.
                        pA = ps1.tile([128, CHUNK], f32, tag="p1", name="pA")
                        pB = ps1.tile([128, CHUNK], f32, tag="p1", name="pB")
                        for k in range(KF):
                            for mi, pp in ((0, pA), (1, pB)):
                                for ti, (wt, xs_) in enumerate((
                                    (w1h_sb, xh), (w1l_sb, xh), (w1h_sb, xl),
                                )):
                                    nc.tensor.matmul(
                                        pp[:, :n],
                                        lhsT=wt[:, k, mi * 128:(mi + 1) * 128],
                                        rhs=xs_[:, k, :n],
                                        start=(k == 0 and ti == 0),
                                        stop=False,
                                    )
                        for mi, pp in ((0, pA), (1, pB)):
                            nc.tensor.matmul(
                                pp[:, :n],
                                lhsT=w1t_sb[:, mi * 128:(mi + 1) * 128],
                                rhs=xt[:, :n],
                                start=False,
                                stop=True,
                            )
                            evict1(pp, mi)
                        for m in range(2, HC):
                            m_block(m)
                    else:
                        for m in range(HC):
                            m_block(m)
                else:
                    # Short tail block (n=128): flip the orientation — x is
                    # stationary, w streams at N=512 — then transpose back.
                    for half in range(2):
                        p1 = ps1.tile([128, CHUNK], f32, tag="p1")
                        hs = slice(half * 512, (half + 1) * 512)
                        i = 0
                        for k in range(KF):
                            for (wt, xs_) in (
                                (w1h_sb, xh), (w1l_sb, xh), (w1h_sb, xl),
                            ):
                                nc.tensor.matmul(
                                    p1[:],
                                    lhsT=xs_[:, k, :n],
                                    rhs=wt[:, k, hs],
                                    start=(i == 0),
                                    stop=False,
                                )
                                i += 1
                        nc.tensor.matmul(
                            p1[:],
                            lhsT=xt[:, :n],
                            rhs=w1t_sb[:, hs],
                            start=False,
                            stop=True,
                        )
                        nc.scalar.activation(
                            out=ttmp[:, hs], in_=p1[:],
                            func=Act.Copy, bias=0.0, scale=1.0,
                        )
                    s = subs[0]
                    nt = n // BL
                    for m in range(HC):
                        pt = ps2.tile([128, 128], f32, tag="pt")
                        nc.tensor.transpose(
                            pt[:], ttmp[:, m * 128:(m + 1) * 128], ident[:]
                        )
                        ptv = pt.rearrange("p (t b) -> p t b", b=BL)
                        nc.scalar.activation(
                            out=c1_tiles[s][:, :nt, m, :],
                            in_=ptv[:, :nt, :],
                            func=Act.Identity,
                            bias=b1_sb[:, m:m + 1],
                            scale=1.0,
                        )

            def gemm2(s):
                # layer-2 GEMM for one sub-block; the last full sub-block is
                # time-halved so the layer-2 scan can start while layer-1's
                # scan of the final block is still draining.
                c0, n = blocks[s]
                nt = n // BL
                c1_tiles.pop(s)
                spk = spk_tiles.pop(s)
                if s != NSUB - 2:
                    halves = ((0, nt),)
                else:
                    q = nt // 4
                    halves = tuple((i * q, (i + 1) * q) for i in range(4))
                for (t0, t1) in halves:
                    s0, s1 = t0 * BL, t1 * BL
                    p2 = ps2.tile([W2W, CHUNK], f32, tag="p2")
                    for c in range(HC):
                        nc.tensor.matmul(
                            p2[:, s0:s1],
                            lhsT=w2p_sb[:, c, :],
                            rhs=spk[:, c, t0:t1, :],
                            start=(c == 0),
                            stop=(c == HC - 1),
                        )
                    # strip combine: c2 = (psum[0:10] + b2) + psum[32:42]
                    nc.scalar.activation(
                        out=c2[:, c0 + s0:c0 + s1],
                        in_=p2[0:O, s0:s1],
                        func=Act.Identity,
                        bias=b2_sb[:, 0:1],
                        scale=1.0,
                    )
                    nc.vector.scalar_tensor_tensor(
                        out=c2[:, c0 + s0:c0 + s1], in0=p2[32:32 + O, s0:s1],
                        scalar=1.0, in1=c2[:, c0 + s0:c0 + s1],
                        op0=Alu.mult, op1=Alu.add,
                    )

            def emit_scans(s):
                # scan1(s) interleaved op-by-op with scan2(s-1) so the
                # same-engine RAW semaphore waits hide behind the other
                # chain; then the blockwise spike extraction for s.
                nonlocal v1, v2
                c0, n = blocks[s]
                nt1 = n // BL
                c1 = c1_tiles[s]
                prev = s - 1
                if prev >= 0:
                    p0, pn = blocks[prev]
                    nt2 = pn // BL
                else:
                    nt2 = 0
                for j in range(max(nt1, nt2)):
                    if j < nt1:
                        csf = c1[:, j].rearrange("p c b -> p (c b)")
                        nc.vector.scalar_tensor_tensor(
                            out=csf, in0=v1[:], scalar=-BETA, in1=csf,
                            op0=Alu.mult, op1=Alu.add,
                        )
                    if j < nt2:
                        t = p0 // BL + j
                        ms = c2[:, t * BL:(t + 1) * BL]
                        nc.vector.scalar_tensor_tensor(
                            out=ms, in0=v2[:], scalar=-BETA, in1=ms,
                            op0=Alu.mult, op1=Alu.add,
                        )
                    if j < nt1:
                        v1n = mvpool.tile([128, HC * BL], f32, tag="v1")
                        nc.vector.scalar_tensor_tensor(
                            out=v1n[:], in0=csf, scalar=BETA, in1=csf,
                            op0=Alu.is_gt, op1=Alu.subtract,
                        )
                        v1 = v1n
                    if j < nt2:
                        t = p0 // BL + j
                        ms = c2[:, t * BL:(t + 1) * BL]
                        v2n = mvpool.tile([O, BL], f32, tag="v2")
                        nc.vector.scalar_tensor_tensor(
                            out=v2n[:], in0=ms, scalar=BETA, in1=ms,
                            op0=Alu.is_gt, op1=Alu.subtract,
                        )
                        v2 = v2n
                    if s == NSUB - 2 and j in (7, 15, 23):
                        # quarter spikes early, for the time-quartered gemm2
                        spk5 = spk_tiles[s]
                        for cix in range(HC):
                            nc.vector.tensor_scalar(
                                spk5[:, cix, j - 7:j + 1, :],
                                c1[:, j - 7:j + 1, cix, :],
                                BETA, None, Alu.is_gt,
                            )
                        extract_ranges[s] = ((j + 1, nt1),)
                # blockwise spikes for s: spk = (M > beta) in {0,1}; one
                # DVE op per h-chunk, queued right after the scans they need
                spk = spk_tiles[s]
                for (t0, t1) in extract_ranges.pop(s, ((0, nt1),)):
                    for cix in range(HC):
                        nc.vector.tensor_scalar(
                            spk[:, cix, t0:t1, :],
                            c1[:, t0:t1, cix, :],
                            BETA, None, Alu.is_gt,
                        )
                if prev >= 0:
                    s2b = mvpool.tile([O, CHUNK], f32, tag="s2b")
                    nc.vector.tensor_scalar(
                        s2b[:, :pn], c2[:, p0:p0 + pn], BETA, None, Alu.is_gt,
                    )
                    pending_out[prev] = s2b

            def emit_out(s):
                # S2/M2 DMAs for sub-block s; emitted late so they never sit
                # in front of input x DMAs in the in-order Sync queue.
                c0, n = blocks[s]
                s2b = pending_out.pop(s)
                nc.sync.dma_start(out=S2[:, c0:c0 + n], in_=s2b[:, :n])
                nc.sync.dma_start(out=M2[:, c0:c0 + n], in_=c2[:, c0:c0 + n])

            # Software pipeline (round r):
            #   PE:   gemm1(r) then gemm2(r-1)  -> scan/spike of r-1 get a
            #         full gemm1 window to finish, PE never waits on them.
            #   DVE:  scan1(r) + scan2(r-1), interleaved.
            #   Sync: x DMAs for r (in gemm1), then outputs of r-2.
            for bi in range(NSUB):
                gemm1(bi)
                if bi == 0:
                    nc.sync.dma_start(out=w2p_sb[:], in_=w2p_d[:])
                    nc.sync.dma_start(out=b2_sb[:], in_=b2c[:])
                if bi > 0:
                    gemm2(bi - 1)
                emit_scans(bi)
                if bi >= 2:
                    emit_out(bi - 2)
            gemm2(NSUB - 1)
            emit_out(NSUB - 2)
            # drain: scan2 for the last sub-block
            for sd in (NSUB - 1,):
                c0l, nl = blocks[sd]
                ntl = nl // BL
                for j in range(ntl):
                    t = c0l // BL + j
                    ms = c2[:, t * BL:(t + 1) * BL]
                    nc.vector.scalar_tensor_tensor(
                        out=ms, in0=v2[:], scalar=-BETA, in1=ms,
                        op0=Alu.mult, op1=Alu.add,
                    )
                    v2n = mvpool.tile([O, BL], f32, tag="v2", name="v2n")
                    nc.vector.scalar_tensor_tensor(
                        out=v2n[:], in0=ms, scalar=BETA, in1=ms,
                        op0=Alu.is_gt, op1=Alu.subtract,
                    )
                    v2 = v2n
                s2b = mvpool.tile([O, CHUNK], f32, tag="s2b", name="s2b")
                nc.vector.tensor_scalar(
                    s2b[:, :nl], c2[:, c0l:c0l + nl], BETA, None, Alu.is_gt,
                )
                pending_out[sd] = s2b
                emit_out(sd)

    nc.compile()
    return nc


def _get_nc():
    global _nc_cache
    if _nc_cache is None:
        _nc_cache = _build()
    return _nc_cache


def _f16(a):
    return np.asarray(a, np.float16)


def _split16(a):
    hi = _f16(a)
    lo = _f16(np.asarray(a, np.float32) - hi.astype(np.float32))
    return hi, lo


def _prep_shared(w1, b1, w2, b2):
    w1s = (BETA * w1).T.astype(np.float32)        # (784, 1024)
    w1h_f, w1l_f = _split16(w1s)
    w1h = np.ascontiguousarray(w1h_f[:768])
    w1l = np.ascontiguousarray(w1l_f[:768])
    # packed 48-row tail: pairs (w1h,xh), (w1l,xh), (w1h,xl) in one matmul
    w1t = np.ascontiguousarray(
        np.concatenate([w1h_f[768:], w1l_f[768:], w1h_f[768:]], axis=0)
    )
    b1c = np.ascontiguousarray((BETA * b1).astype(np.float32).reshape(HC, 128).T)
    # GEMM2 consumes 0/1 spikes; stationary packs w2h at 0..9, w2l at 32..41
    w2s = (BETA * w2).T.astype(np.float32).reshape(HC, 128, O).transpose(1, 0, 2)
    w2h, w2l = _split16(np.ascontiguousarray(w2s))    # (128, HC, O) each
    w2p_a = np.zeros((128, HC, W2W), np.float16)
    w2p_a[:, :, :O] = w2h
    w2p_a[:, :, 32:32 + O] = w2l
    w2p = np.ascontiguousarray(w2p_a.reshape(128, HC * W2W))
    b2c = (BETA * b2).astype(np.float32).reshape(O, 1)
    return w1h, w1l, w1t, b1c, w2p, b2c


def _make_in_maps(x, w1, b1, w2, b2):
    w1h, w1l, w1t, b1c, w2p, b2c = _prep_shared(w1, b1, w2, b2)
    in_maps = []
    for c in range(NCORES):
        xs = x[c * BL:(c + 1) * BL]                     # (BL, T, I)
        xT = np.ascontiguousarray(
            xs.transpose(2, 1, 0).reshape(I, TB)        # col = t*BL + b
        )
        xh_f, xl_f = _split16(xT)
        xh = np.ascontiguousarray(xh_f[:768])
        xl = np.ascontiguousarray(xl_f[:768])
        xt = np.ascontiguousarray(
            np.concatenate([xh_f[768:], xh_f[768:], xl_f[768:]], axis=0)
        )
        in_maps.append({
            "xh": xh, "xl": xl, "xt": xt, "w1h": w1h, "w1l": w1l, "w1t": w1t,
            "b1c": b1c, "w2p": w2p, "b2c": b2c,
        })
    return in_maps


def kernel(x, w1, b1, w2, b2):
    from concourse.bass_utils import run_bass_kernel_spmd

    nc = _get_nc()
    in_maps = _make_in_maps(x, w1, b1, w2, b2)
    res = run_bass_kernel_spmd(nc, in_maps, core_ids=list(range(NCORES)))

    spk = np.empty((B, T, O), np.float32)
    mem = np.empty((B, T, O), np.float32)
    for c in range(NCORES):
        r = res.results[c]
        spk[c * BL:(c + 1) * BL] = r["S2"].reshape(O, T, BL).transpose(2, 1, 0)
        mem[c * BL:(c + 1) * BL] = (
            r["M2"].reshape(O, T, BL).transpose(2, 1, 0) * np.float32(1.0 / BETA)
        )
    return spk, mem
